# revision 4
# baseline (speedup 1.0000x reference)
"""Trainium2 Bass kernel for nn_Dynamic_deformable_DySample_restart.

Problem: 3x3 conv (30->84ch) over guidance produces per-pixel offsets +
softmax affinities for 3 iterations of a modulated deformable 3x3 conv
(bilinear sampling) with restart/confidence blending.

Strategy (8 NeuronCores, pure data parallel, one NEFF):
  - shard = (batch b, H-half) -> 8 shards of 176 output rows (+ margins).
  - Phase 1 (PE): conv as 3 accumulating matmuls (K=90 = 30ch x 3kx taps,
    kx realized as column-shifted loads of bf16 guidance), back-to-back to
    keep the PE in its high p-state; PSUM evacuation split across ACT
    (exp of softmax logits, bias folded) and DVE/GPSIMD (offset bias-add),
    fields spilled to internal DRAM as fp16.
  - Phase 2 (DVE+GPSIMD+ACT): per iteration, per 64-row x 2-half band:
    feat as a row-duplicated slab [P, 5, 612] fp16 (halo rows in the free
    dim). Bilinear sample via the 3-candidate hat identity per axis
      G(row) = f + dx*Dp(b-1) + relu(dx)*D2(b)        (x interp)
      val    = G1 + dy*(G1-G0) + relu(dy)*(G2-2G1+G0) (y interp)
    batched over all (ky, r) row-candidates per kx column group as single
    wide instructions ([P, 3, 3, 608] APs with broadcast dims), then the
    y-combine batched over all 9 taps at once ([P, 3, 3, 608]).
    Each wide op is column-split DVE | GPSIMD to balance both engines.
    Everything fp16 (2x DVE mode); relu fields produced on ACT.
  - Zero-padding at image borders carried in the data (host-padded inputs;
    off-image rows masked via om_conf/conf_ff).
"""
import os
import numpy as np
import ml_dtypes
from contextlib import ExitStack

import concourse.bacc as bacc
import concourse.bass as bass
import concourse.tile as tile
import concourse.mybir as mybir
from concourse.bass_utils import run_bass_kernel_spmd

F32 = mybir.dt.float32
F16 = mybir.dt.float16
BF16 = mybir.dt.bfloat16
ALU = mybir.AluOpType
AF = mybir.ActivationFunctionType

# ---------------- geometry ----------------
B, H, W = 4, 352, 1216
HALF = 176               # output rows per core
NC = 8
C0 = HALF + 8            # 184: rows where fields/iter-0 feat are computed
GR = C0 + 2              # 186: guidance rows needed (conv halo)
FR = C0 + 4              # 188: feat rows (init + buffer)
WG = W + 2               # 1218: guidance cols incl conv pad
WF = W + 4               # 1220: feat cols incl +-2 pad
CH = 8                   # conv row-chunk
NCHUNK = C0 // CH        # 23
NT = 19                  # 512-px tiles per chunk (8 rows x 64 cols)
W2 = W // 2              # 608 col half
FS = C0 * W              # field plane stride (184*1216)
SPL = 440                # G/y ops: cols [0,SPL) on DVE, [SPL,608) on GPSIMD

# conv output channel order (M = 94):
#  offsets occupy m 0..31 and 35..56 (pos = 18*k + idx; m = pos if pos<32
#  else pos+3); m 32..34 are junk; m 64..93: logits (exp reads at base 64);
#  m 57..63 pad.
MM = 94

_CACHE = {}


def _dap(t, offset, dims):
    return bass.AP(tensor=t, offset=offset, ap=[list(d) for d in dims])


def _sv(t, off, dims):
    """Custom strided view of an SBUF tile AP; partition dim preserved."""
    return bass.AP(tensor=t.tensor, offset=t.offset + off,
                   ap=[list(t.ap[0])] + [list(d) for d in dims])


def _svp(t, P, off, dims):
    """Like _sv but with partition count P."""
    return bass.AP(tensor=t.tensor, offset=t.offset + off,
                   ap=[[t.ap[0][0], P]] + [list(d) for d in dims])


def _build_program(do_p1=True, do_p2=True):
    nc = bacc.Bacc("TRN2", target_bir_lowering=False, debug=False)

    g_d = nc.dram_tensor("g", [30, GR, WG], BF16, kind="ExternalInput")
    w3_d = nc.dram_tensor("w3", [90, 3, MM], BF16, kind="ExternalInput")
    b94_d = nc.dram_tensor("b94", [MM, 1], F32, kind="ExternalInput")
    blog_d = nc.dram_tensor("blog", [30, 1], F32, kind="ExternalInput")
    fin_d = nc.dram_tensor("finit", [FR, WF], F16, kind="ExternalInput")
    omc_d = nc.dram_tensor("omc", [C0, W], F16, kind="ExternalInput")
    cff_d = nc.dram_tensor("cff", [C0, W], F16, kind="ExternalInput")
    out_d = nc.dram_tensor("out", [HALF, W], F32, kind="ExternalOutput")

    featbuf_a = nc.dram_tensor("featbuf_a", [FR, WF], F16, kind="Internal")
    featbuf_b = nc.dram_tensor("featbuf_b", [FR, WF], F16, kind="Internal")
    offs_d = nc.dram_tensor("offs", [3, 18, C0, W], F16, kind="Internal")
    es_d = nc.dram_tensor("es", [3, 10, C0, W], F16, kind="Internal")

    with tile.TileContext(nc) as tc, ExitStack() as octx:
        # ---- persistent small tiles ----
        singles = octx.enter_context(tc.tile_pool(name="singles", bufs=1))
        w3_sb = singles.tile([90, 3, MM], BF16, tag="w3")
        nc.sync.dma_start(out=w3_sb, in_=w3_d.ap())
        b94_sb = singles.tile([MM, 1], F32, tag="b94")
        nc.sync.dma_start(out=b94_sb, in_=b94_d.ap())
        blog_sb = singles.tile([30, 1], F32, tag="blog")
        nc.sync.dma_start(out=blog_sb, in_=blog_d.ap())
        zt = singles.tile([1, 2 * FR], F16, tag="zt")
        nc.vector.memset(zt, 0.0)
        # zero the feat-buffer column pads (rows never write cols [0,2)+[1218,1220))
        for fb in (featbuf_a, featbuf_b):
            nc.sync.dma_start(out=_dap(fb, 0, [[WF, FR], [1, 2]]),
                              in_=zt[:, 0:2 * FR])
            nc.sync.dma_start(out=_dap(fb, W + 2, [[WF, FR], [1, 2]]),
                              in_=zt[:, 0:2 * FR])

        # ================= Phase 1: conv + field extraction =================
        with ExitStack() as ctx:
            g3p = ctx.enter_context(tc.tile_pool(name="g3", bufs=2))
            stp = ctx.enter_context(tc.tile_pool(name="stage", bufs=2))
            pp = ctx.enter_context(tc.tile_pool(name="psA", bufs=6, space="PSUM"))

            for ci in range(NCHUNK if do_p1 else 0):
                g3 = g3p.tile([90, CH + 2, W], BF16, tag="g3")
                for kx in range(3):
                    nc.sync.dma_start(
                        out=g3[30 * kx:30 * kx + 30],
                        in_=_dap(g_d, (ci * CH) * WG + kx,
                                 [[GR * WG, 30], [WG, CH + 2], [1, W]]))
                all_st = stp.tile([57, CH, NT * 64], F16, tag="all_st")
                e_st = stp.tile([30, CH, NT * 64], F16, tag="e_st")
                for ti in range(NT):
                    pa = pp.tile([MM, 512], F32, tag="pa")
                    for ky in range(3):
                        nc.tensor.matmul(
                            pa[0:MM], w3_sb[:, ky],
                            g3[:, ky:ky + CH, ti * 64:(ti + 1) * 64],
                            start=(ky == 0), stop=(ky == 2))
                    nc.scalar.activation(
                        out=e_st[:, :, ti * 64:(ti + 1) * 64], in_=pa[64:94],
                        func=AF.Exp, bias=blog_sb, scale=1.0)
                    # offsets(+bias) rows 0..31+35..56 (rows 32..34 junk)
                    nc.vector.tensor_scalar(
                        out=all_st[:, :, ti * 64:(ti + 1) * 64], in0=pa[0:57],
                        scalar1=b94_sb[0:57], scalar2=None, op0=ALU.add)
                # spill chunk fields to DRAM
                ro = ci * CH * W
                nc.sync.dma_start(
                    out=_dap(offs_d, ro, [[FS, 32], [W, CH], [1, W]]), in_=all_st[0:32])
                nc.sync.dma_start(
                    out=_dap(offs_d, 32 * FS + ro, [[FS, 22], [W, CH], [1, W]]),
                    in_=all_st[35:57])
                nc.sync.dma_start(
                    out=_dap(es_d, ro, [[FS, 30], [W, CH], [1, W]]), in_=e_st)

        # ================= Phase 2: deformable iterations =================
        with ExitStack() as ctx:
            slabp = ctx.enter_context(tc.tile_pool(name="slab", bufs=2))
            dpp = ctx.enter_context(tc.tile_pool(name="dp", bufs=1))
            gtp = ctx.enter_context(tc.tile_pool(name="gt", bufs=1))
            fldp = ctx.enter_context(tc.tile_pool(name="fld", bufs=2))
            scrp = ctx.enter_context(tc.tile_pool(name="scr", bufs=1))
            accp = ctx.enter_context(tc.tile_pool(name="acc", bufs=2))

            for k in range(3 if do_p2 else 0):
                rk = C0 - 4 * k
                src_d = fin_d if k == 0 else (featbuf_a if k == 1 else featbuf_b)
                dst_fb = featbuf_a if k == 0 else featbuf_b
                for (lo, rows) in ((0, 64), (64, 64), (128, rk - 128)):
                    P = 2 * rows
                    ro = (2 * k + lo) * W          # field row offset
                    slab = slabp.tile([128, 5, 612], F16, tag="slab")
                    for h in range(2):
                        nc.sync.dma_start(
                            out=slab[h * rows:(h + 1) * rows],
                            in_=_dap(src_d, (lo + 2 * k) * WF + W2 * h,
                                     [[WF, rows], [WF, 5], [1, 612]]))
                    # slab1[c] = slab[c+1]: 4B-aligned mirror for odd column shifts
                    slab1 = slabp.tile([128, 5, 612], F16, tag="slab1")
                    nc.sync.dma_start(out=slab1[0:P, :, 0:611], in_=slab[0:P, :, 1:612])

                    # field loads
                    def ldfield(dst, base_d, off0, nf=1):
                        for h in range(2):
                            dims = [[W, rows]] + ([[FS, nf]] if nf > 1 else []) + [[1, W2]]
                            nc.sync.dma_start(
                                out=dst[h * rows:(h + 1) * rows],
                                in_=_dap(base_d, off0 + ro + W2 * h, dims))

                    omc_t = fldp.tile([128, W2], F16, tag="omc", bufs=1)
                    ldfield(omc_t, omc_d, 0)
                    cff_t = fldp.tile([128, W2], F16, tag="cff", bufs=1)
                    ldfield(cff_t, cff_d, 0)
                    offt = fldp.tile([128, 18, W2], F16, tag="offt")
                    ldfield(offt, offs_d, k * 18 * FS, nf=18)
                    est = fldp.tile([128, 10, W2], F16, tag="est", bufs=1)
                    ldfield(est, es_d, k * 10 * FS, nf=10)

                    # x-differences (both column parities)
                    dpa = dpp.tile([128, 5, 612], F16, tag="dpa")
                    dpb = dpp.tile([128, 5, 612], F16, tag="dpb")
                    nc.vector.tensor_tensor(out=dpa[0:P, :, 0:611], in0=slab1[0:P, :, 0:611],
                                            in1=slab[0:P, :, 0:611], op=ALU.subtract)
                    nc.vector.tensor_tensor(out=dpb[0:P, :, 0:610], in0=slab[0:P, :, 2:612],
                                            in1=slab1[0:P, :, 0:610], op=ALU.subtract)
                    d2a = dpp.tile([128, 5, 612], F16, tag="d2a")
                    d2b = dpp.tile([128, 5, 612], F16, tag="d2b")
                    nc.vector.tensor_tensor(out=d2a[0:P, :, 2:611], in0=dpa[0:P, :, 2:611],
                                            in1=dpb[0:P, :, 0:609], op=ALU.subtract)
                    nc.vector.tensor_tensor(out=d2b[0:P, :, 0:610], in0=dpb[0:P, :, 0:610],
                                            in1=dpa[0:P, :, 0:610], op=ALU.subtract)

                    # relu fields on ACT: rdy9[t]=relu(dy_t), rdx9[t]=relu(dx_t)
                    rdy9 = scrp.tile([128, 9, W2], F16, tag="rdy9")
                    nc.scalar.activation(
                        out=rdy9[0:P], in_=_svp(offt, P, 0, [[2 * W2, 9], [1, W2]]),
                        func=AF.Relu)
                    rdx9 = scrp.tile([128, 9, W2], F16, tag="rdx9")
                    nc.scalar.activation(
                        out=rdx9[0:P], in_=_svp(offt, P, W2, [[2 * W2, 9], [1, W2]]),
                        func=AF.Relu)

                    # ---- G stage: per kx, batched over (ky, r) ----
                    # G[kx, ky, r] = slab(ky+r, b) + dx*Dp(b-1) + rdx*D2(b)
                    Gt = gtp.tile([128, 3, 3, 3, W2], F16, tag="Gt")  # [kx][ky][r]
                    t2t = scrp.tile([128, 3, 3, W2], F16, tag="t2t")
                    for kx in range(3):
                        bb = kx - 1
                        sl_t, sl_o = (slab, 2 + bb) if (2 + bb) % 2 == 0 else (slab1, 1 + bb)
                        dp_t, dp_o = (dpa, 1 + bb) if (1 + bb) % 2 == 0 else (dpb, bb)
                        d2_t, d2_o = (d2a, 2 + bb) if (2 + bb) % 2 == 0 else (d2b, 1 + bb)
                        kyr = [[612, 3], [612, 3], [1, W2]]
                        slv = _svp(sl_t, P, sl_o, kyr)
                        dpv = _svp(dp_t, P, dp_o, kyr)
                        d2v = _svp(d2_t, P, d2_o, kyr)
                        dxv = _svp(offt, P, (2 * kx + 1) * W2,
                                   [[6 * W2, 3], [0, 3], [1, W2]])
                        rxv = _svp(rdx9, P, kx * W2,
                                   [[3 * W2, 3], [0, 3], [1, W2]])
                        gv = _svp(Gt, P, kx * 9 * W2,
                                  [[3 * W2, 3], [W2, 3], [1, W2]])
                        t2v = _svp(t2t, P, 0, [[3 * W2, 3], [W2, 3], [1, W2]])
                        # column-split DVE | GPSIMD
                        for (eng, c0, cn) in ((nc.vector, 0, SPL),
                                              (nc.gpsimd, SPL, W2 - SPL)):
                            def cs(v):
                                return bass.AP(tensor=v.tensor, offset=v.offset + c0,
                                               ap=[list(v.ap[0])] +
                                                  [list(d) for d in v.ap[1:-1]] +
                                                  [[1, cn]])
                            eng.tensor_tensor(out=cs(gv), in0=cs(rxv), in1=cs(d2v), op=ALU.mult)
                            eng.tensor_tensor(out=cs(t2v), in0=cs(dxv), in1=cs(dpv), op=ALU.mult)
                            eng.tensor_tensor(out=cs(gv), in0=cs(gv), in1=cs(t2v), op=ALU.add)
                            eng.tensor_tensor(out=cs(gv), in0=cs(gv), in1=cs(slv), op=ALU.add)

                    # ---- y combine: batched over all 9 taps (kx, ky) ----
                    # val = G1 + dy*(G1-G0) + rdy*((G2-G1)-(G1-G0))
                    dY = t2t          # t2t is dead after the G stage
                    T2 = scrp.tile([128, 9, W2], F16, tag="T2")
                    val = rdx9        # rdx9 is dead after the G stage
                    kk = [[9 * W2, 3], [3 * W2, 3], [1, W2]]   # (kx, ky) dims on Gt
                    g0 = _svp(Gt, P, 0, kk)
                    g1 = _svp(Gt, P, W2, kk)
                    g2 = _svp(Gt, P, 2 * W2, kk)
                    dyv = _svp(offt, P, 0, [[2 * W2, 3], [6 * W2, 3], [1, W2]])
                    ryv = _svp(rdy9, P, 0, [[W2, 3], [3 * W2, 3], [1, W2]])
                    flat9 = [[W2, 9], [1, W2]]
                    dYv = _svp(dY, P, 0, [[3 * W2, 3], [W2, 3], [1, W2]])
                    T2v = _svp(T2, P, 0, [[3 * W2, 3], [W2, 3], [1, W2]])
                    valv = _svp(val, P, 0, [[3 * W2, 3], [W2, 3], [1, W2]])
                    for (eng, c0, cn) in ((nc.vector, 0, SPL),
                                          (nc.gpsimd, SPL, W2 - SPL)):
                        def cs(v):
                            return bass.AP(tensor=v.tensor, offset=v.offset + c0,
                                           ap=[list(v.ap[0])] +
                                              [list(d) for d in v.ap[1:-1]] +
                                              [[1, cn]])
                        eng.tensor_tensor(out=cs(dYv), in0=cs(g1), in1=cs(g0), op=ALU.subtract)
                        eng.tensor_tensor(out=cs(T2v), in0=cs(g2), in1=cs(g1), op=ALU.subtract)
                        eng.tensor_tensor(out=cs(T2v), in0=cs(T2v), in1=cs(dYv), op=ALU.subtract)
                        eng.tensor_tensor(out=cs(dYv), in0=cs(dyv), in1=cs(dYv), op=ALU.mult)
                        eng.tensor_tensor(out=cs(T2v), in0=cs(ryv), in1=cs(T2v), op=ALU.mult)
                        eng.tensor_tensor(out=cs(valv), in0=cs(g1), in1=cs(dYv), op=ALU.add)
                        eng.tensor_tensor(out=cs(valv), in0=cs(valv), in1=cs(T2v), op=ALU.add)
                        # ev = e * val  (e reordered to (kx, ky) to match val)
                        ev = _svp(est, P, 0, [[W2, 3], [3 * W2, 3], [1, W2]])
                        eng.tensor_tensor(out=cs(valv), in0=cs(ev), in1=cs(valv), op=ALU.mult)

                    # tap accumulation tree (9 -> 1) + restart term
                    s1 = scrp.tile([128, 4, W2], F16, tag="s1")
                    nc.vector.tensor_tensor(out=s1[0:P], in0=val[0:P, 0:4],
                                            in1=val[0:P, 4:8], op=ALU.add)
                    nc.vector.tensor_tensor(out=s1[0:P, 0:2], in0=s1[0:P, 0:2],
                                            in1=s1[0:P, 2:4], op=ALU.add)
                    prop = accp.tile([128, W2], F16, tag="prop")
                    nc.vector.tensor_tensor(out=prop[0:P], in0=s1[0:P, 0],
                                            in1=s1[0:P, 1], op=ALU.add)
                    nc.vector.tensor_tensor(out=prop[0:P], in0=prop[0:P],
                                            in1=val[0:P, 8], op=ALU.add)
                    # restart: += e9 * feat_center
                    tfe = scrp.tile([128, W2], F16, tag="tfe")
                    nc.gpsimd.tensor_tensor(out=tfe[0:P], in0=est[0:P, 9],
                                            in1=slab[0:P, 2, 2:2 + W2], op=ALU.mult)
                    nc.vector.tensor_tensor(out=prop[0:P], in0=prop[0:P],
                                            in1=tfe[0:P], op=ALU.add)

                    # softmax denominator S = sum est[0:10]; scale = omc/S
                    s5 = scrp.tile([128, 5, W2], F16, tag="s5")
                    nc.gpsimd.tensor_tensor(out=s5[0:P], in0=est[0:P, 0:5],
                                            in1=est[0:P, 5:10], op=ALU.add)
                    nc.gpsimd.tensor_tensor(out=s5[0:P, 0:2], in0=s5[0:P, 0:2],
                                            in1=s5[0:P, 2:4], op=ALU.add)
                    nc.gpsimd.tensor_tensor(out=s5[0:P, 0], in0=s5[0:P, 0],
                                            in1=s5[0:P, 1], op=ALU.add)
                    nc.gpsimd.tensor_tensor(out=s5[0:P, 0], in0=s5[0:P, 0],
                                            in1=s5[0:P, 4], op=ALU.add)
                    rs_t = scrp.tile([128, W2], F32, tag="rs")
                    nc.vector.reciprocal(out=rs_t[0:P], in_=s5[0:P, 0])
                    rs16 = scrp.tile([128, W2], F16, tag="rs16")
                    nc.vector.tensor_copy(out=rs16[0:P], in_=rs_t[0:P])
                    omcrs = scrp.tile([128, W2], F16, tag="omcrs")
                    nc.gpsimd.tensor_tensor(out=omcrs[0:P], in0=omc_t[0:P],
                                            in1=rs16[0:P], op=ALU.mult)

                    # blend: fnew = prop*omc/S + conf*feat_fix
                    nc.vector.tensor_tensor(out=prop[0:P], in0=prop[0:P],
                                            in1=omcrs[0:P], op=ALU.mult)
                    fnew = accp.tile([128, W2], F32 if k == 2 else F16,
                                     tag="fnew32" if k == 2 else "fnew16")
                    nc.vector.tensor_tensor(out=fnew[0:P], in0=prop[0:P],
                                            in1=cff_t[0:P], op=ALU.add)
                    for h in range(2):
                        if k < 2:
                            dst = _dap(dst_fb, (2 + 2 * k + lo) * WF + 2 + W2 * h,
                                       [[WF, rows], [1, W2]])
                        else:
                            dst = _dap(out_d, lo * W + W2 * h, [[W, rows], [1, W2]])
                        nc.sync.dma_start(out=dst, in_=fnew[h * rows:(h + 1) * rows])

    nc.compile()
    return nc


def _prep_inputs(inputs):
    """Full inputs -> list of 8 per-core input dicts (host-side shard+pad)."""
    feat_init = np.asarray(inputs["feat_init"], np.float32)
    guidance = np.asarray(inputs["guidance"], np.float32)
    confidence = np.asarray(inputs["confidence"], np.float32)
    feat_fix = np.asarray(inputs["feat_fix"], np.float32)
    W_conv = np.asarray(inputs["W_conv"], np.float32)
    b_conv = np.asarray(inputs["b_conv"], np.float32)

    # channel reorder: original channel o -> (k = o//28, idx = o%28)
    perm_m = np.zeros(84, np.int64)
    bias94 = np.zeros((MM, 1), np.float32)
    for o in range(84):
        k, idx = o // 28, o % 28
        if idx < 18:
            pos = 18 * k + idx
            m = pos if pos < 32 else pos + 3
        else:
            m = 64 + 10 * k + (idx - 18)
        perm_m[o] = m
        bias94[m, 0] = b_conv[o]
    w3 = np.zeros((90, 3, MM), np.float32)
    for o in range(84):
        for c in range(30):
            for ky in range(3):
                for kx in range(3):
                    w3[kx * 30 + c, ky, perm_m[o]] = W_conv[o, c, ky, kx]
    w3 = w3.astype(ml_dtypes.bfloat16)
    conf = np.sign(feat_fix) * (1.0 / (1.0 + np.exp(-confidence)))
    omc_full = (1.0 - conf)[:, 0].astype(np.float32)     # [B,H,W]
    cff_full = (conf * feat_fix)[:, 0].astype(np.float32)

    def pad_rows(img, lo, hi, fill=0.0):
        """rows [lo, hi) of img [H, ...] with zero padding outside."""
        out = np.full((hi - lo,) + img.shape[1:], fill, img.dtype)
        s0, s1 = max(lo, 0), min(hi, H)
        out[s0 - lo:s1 - lo] = img[s0:s1]
        return out

    in_maps = []
    for core in range(NC):
        b, half = core // 2, core % 2
        r0 = half * HALF
        g_sh = np.zeros((30, GR, WG), np.float32)
        glo, ghi = r0 - 5, r0 + HALF + 5
        s0, s1 = max(glo, 0), min(ghi, H)
        g_sh[:, s0 - glo:s1 - glo, 1:W + 1] = guidance[b, :, s0:s1, :]
        f_sh = np.zeros((FR, WF), np.float32)
        flo, fhi = r0 - 6, r0 + HALF + 6
        s0, s1 = max(flo, 0), min(fhi, H)
        f_sh[s0 - flo:s1 - flo, 2:W + 2] = feat_init[b, 0, s0:s1, :]
        in_maps.append({
            "g": g_sh.astype(ml_dtypes.bfloat16),
            "w3": w3,
            "b94": bias94,
            "blog": np.ascontiguousarray(bias94[64:94]),
            "finit": f_sh.astype(np.float16),
            "omc": np.ascontiguousarray(pad_rows(omc_full[b], r0 - 4, r0 + HALF + 4)).astype(np.float16),
            "cff": np.ascontiguousarray(pad_rows(cff_full[b], r0 - 4, r0 + HALF + 4)).astype(np.float16),
        })
    return in_maps


def kernel(**inputs) -> np.ndarray:
    if "nc" not in _CACHE:
        _CACHE["nc"] = _build_program()
    nc = _CACHE["nc"]
    in_maps = _prep_inputs(inputs)
    trace = os.environ.get("KERNEL_TRACE", "0") == "1"
    res = run_bass_kernel_spmd(nc, in_maps, core_ids=list(range(NC)), trace=trace)
    _CACHE["last_result"] = res
    out = np.zeros((B, 1, H, W), np.float32)
    for core in range(NC):
        b, half = core // 2, core % 2
        out[b, 0, half * HALF:(half + 1) * HALF, :] = res.results[core]["out"]
    return out


# revision 7
# speedup vs baseline: 1.2909x; 1.2909x over previous
"""Trainium2 Bass kernel for nn_Dynamic_deformable_DySample_restart.

Problem: 3x3 conv (30->84ch) over guidance produces per-pixel offsets +
softmax affinities for 3 iterations of a modulated deformable 3x3 conv
(bilinear sampling) with restart/confidence blending.

Strategy (8 NeuronCores, pure data parallel, one NEFF):
  - shard = (batch b, H-half) -> 8 shards of 176 output rows (+ margins).
  - Phase 1 (PE): conv as 3 accumulating matmuls (K=90 = 30ch x 3kx taps,
    kx realized as column-shifted loads of bf16 guidance), back-to-back to
    keep the PE in its high p-state; PSUM evacuation split across ACT
    (exp of softmax logits, bias folded) and DVE/GPSIMD (offset bias-add),
    fields spilled to internal DRAM as fp16.
  - Phase 2 (DVE+GPSIMD+ACT): per iteration, per 64-row x 2-half band:
    feat as a row-duplicated slab [P, 5, 612] fp16 (halo rows in the free
    dim). Bilinear sample via the 3-candidate hat identity per axis
      G(row) = f + dx*Dp(b-1) + relu(dx)*D2(b)        (x interp)
      val    = G1 + dy*(G1-G0) + relu(dy)*(G2-2G1+G0) (y interp)
    batched over all (ky, r) row-candidates per kx column group as single
    wide instructions ([P, 3, 3, 608] APs with broadcast dims), then the
    y-combine batched over all 9 taps at once ([P, 3, 3, 608]).
    Each wide op is column-split DVE | GPSIMD to balance both engines.
    Everything fp16 (2x DVE mode); relu fields produced on ACT.
  - Zero-padding at image borders carried in the data (host-padded inputs;
    off-image rows masked via om_conf/conf_ff).
"""
import os
import numpy as np
import ml_dtypes
from contextlib import ExitStack

import concourse.bacc as bacc
import concourse.bass as bass
import concourse.tile as tile
import concourse.mybir as mybir
from concourse.bass_utils import run_bass_kernel_spmd

F32 = mybir.dt.float32
F16 = mybir.dt.float16
BF16 = mybir.dt.bfloat16
ALU = mybir.AluOpType
AF = mybir.ActivationFunctionType

# ---------------- geometry ----------------
B, H, W = 4, 352, 1216
HALF = 176               # output rows per core
NC = 8
C0 = HALF + 8            # 184: rows where fields/iter-0 feat are computed
GR = C0 + 2              # 186: guidance rows needed (conv halo)
FR = C0 + 4              # 188: feat rows (init + buffer)
WG = W + 2               # 1218: guidance cols incl conv pad
WF = W + 4               # 1220: feat cols incl +-2 pad
CH = 8                   # conv row-chunk
NCHUNK = C0 // CH        # 23
NT = 19                  # 512-px tiles per chunk (8 rows x 64 cols)
W2 = W // 2              # 608 col half
FS = C0 * W              # field plane stride (184*1216)
SPL = 440                # G/y ops: cols [0,SPL) on DVE, [SPL,608) on GPSIMD

# conv output channel order (M = 94):
#  offsets occupy m 0..31 and 35..56 (pos = 18*k + idx; m = pos if pos<32
#  else pos+3); m 32..34 are junk; m 64..93: logits (exp reads at base 64);
#  m 57..63 pad.
MM = 94

_CACHE = {}


def _dap(t, offset, dims):
    return bass.AP(tensor=t, offset=offset, ap=[list(d) for d in dims])


def _sv(t, off, dims):
    """Custom strided view of an SBUF tile AP; partition dim preserved."""
    return bass.AP(tensor=t.tensor, offset=t.offset + off,
                   ap=[list(t.ap[0])] + [list(d) for d in dims])


def _svp(t, P, off, dims):
    """Like _sv but with partition count P."""
    return bass.AP(tensor=t.tensor, offset=t.offset + off,
                   ap=[[t.ap[0][0], P]] + [list(d) for d in dims])


def _build_program(do_p1=True, do_p2=True):
    nc = bacc.Bacc("TRN2", target_bir_lowering=False, debug=False)

    g_d = nc.dram_tensor("g", [30, GR, WG], BF16, kind="ExternalInput")
    w3_d = nc.dram_tensor("w3", [90, 3, MM], BF16, kind="ExternalInput")
    b94_d = nc.dram_tensor("b94", [MM, 1], F32, kind="ExternalInput")
    blog_d = nc.dram_tensor("blog", [30, 1], F32, kind="ExternalInput")
    fin_d = nc.dram_tensor("finit", [FR, WF], BF16, kind="ExternalInput")
    omc_d = nc.dram_tensor("omc", [C0, W], BF16, kind="ExternalInput")
    cff_d = nc.dram_tensor("cff", [C0, W], BF16, kind="ExternalInput")
    out_d = nc.dram_tensor("out", [HALF, W], F32, kind="ExternalOutput")

    featbuf_a = nc.dram_tensor("featbuf_a", [FR, WF], BF16, kind="Internal")
    featbuf_b = nc.dram_tensor("featbuf_b", [FR, WF], BF16, kind="Internal")
    offs_d = nc.dram_tensor("offs", [3, 18, C0, W], BF16, kind="Internal")
    es_d = nc.dram_tensor("es", [3, 10, C0, W], BF16, kind="Internal")

    with tile.TileContext(nc) as tc, ExitStack() as octx:
        # ---- persistent small tiles ----
        singles = octx.enter_context(tc.tile_pool(name="singles", bufs=1))
        w3_sb = singles.tile([90, 3, MM], BF16, tag="w3")
        nc.sync.dma_start(out=w3_sb, in_=w3_d.ap())
        b94_sb = singles.tile([MM, 1], F32, tag="b94")
        nc.sync.dma_start(out=b94_sb, in_=b94_d.ap())
        blog_sb = singles.tile([30, 1], F32, tag="blog")
        nc.sync.dma_start(out=blog_sb, in_=blog_d.ap())
        zt = singles.tile([1, 2 * FR], BF16, tag="zt")
        nc.vector.memset(zt, 0.0)
        # zero the feat-buffer column pads (rows never write cols [0,2)+[1218,1220))
        for fb in (featbuf_a, featbuf_b):
            nc.sync.dma_start(out=_dap(fb, 0, [[WF, FR], [1, 2]]),
                              in_=zt[:, 0:2 * FR])
            nc.sync.dma_start(out=_dap(fb, W + 2, [[WF, FR], [1, 2]]),
                              in_=zt[:, 0:2 * FR])

        # ================= Phase 1: conv + field extraction =================
        with ExitStack() as ctx:
            g3p = ctx.enter_context(tc.tile_pool(name="g3", bufs=2))
            stp = ctx.enter_context(tc.tile_pool(name="stage", bufs=2))
            pp = ctx.enter_context(tc.tile_pool(name="psA", bufs=6, space="PSUM"))

            for ci in range(NCHUNK if do_p1 else 0):
                g3 = g3p.tile([90, CH + 2, W], BF16, tag="g3")
                for kx in range(3):
                    nc.sync.dma_start(
                        out=g3[30 * kx:30 * kx + 30],
                        in_=_dap(g_d, (ci * CH) * WG + kx,
                                 [[GR * WG, 30], [WG, CH + 2], [1, W]]))
                all_st = stp.tile([57, CH, NT * 64], BF16, tag="all_st")
                e_st = stp.tile([30, CH, NT * 64], BF16, tag="e_st")
                for gb in range(0, NT, 6):
                    tis = list(range(gb, min(gb + 6, NT)))
                    pas = {ti: pp.tile([MM, 512], F32, tag="pa", name=f"pa{ti}")
                           for ti in tis}
                    for ky in range(3):
                        for ti in tis:
                            nc.tensor.matmul(
                                pas[ti][0:MM], w3_sb[:, ky],
                                g3[:, ky:ky + CH, ti * 64:(ti + 1) * 64],
                                start=(ky == 0), stop=(ky == 2))
                    for ti in tis:
                        nc.scalar.activation(
                            out=e_st[:, :, ti * 64:(ti + 1) * 64], in_=pas[ti][64:94],
                            func=AF.Exp, bias=blog_sb, scale=1.0)
                        # offsets(+bias) rows 0..31+35..56 (rows 32..34 junk)
                        nc.vector.tensor_scalar(
                            out=all_st[:, :, ti * 64:(ti + 1) * 64], in0=pas[ti][0:57],
                            scalar1=b94_sb[0:57], scalar2=None, op0=ALU.add)
                # spill chunk fields to DRAM
                ro = ci * CH * W
                nc.sync.dma_start(
                    out=_dap(offs_d, ro, [[FS, 32], [W, CH], [1, W]]), in_=all_st[0:32])
                nc.sync.dma_start(
                    out=_dap(offs_d, 32 * FS + ro, [[FS, 22], [W, CH], [1, W]]),
                    in_=all_st[35:57])
                nc.sync.dma_start(
                    out=_dap(es_d, ro, [[FS, 30], [W, CH], [1, W]]), in_=e_st)

        # ================= Phase 2: deformable iterations =================
        with ExitStack() as ctx:
            slabp = ctx.enter_context(tc.tile_pool(name="slab", bufs=2))
            dpp = ctx.enter_context(tc.tile_pool(name="dp", bufs=1))
            gtp = ctx.enter_context(tc.tile_pool(name="gt", bufs=1))
            fldp = ctx.enter_context(tc.tile_pool(name="fld", bufs=2))
            scrp = ctx.enter_context(tc.tile_pool(name="scr", bufs=1))
            accp = ctx.enter_context(tc.tile_pool(name="acc", bufs=2))

            for k in range(3 if do_p2 else 0):
                rk = C0 - 4 * k
                src_d = fin_d if k == 0 else (featbuf_a if k == 1 else featbuf_b)
                dst_fb = featbuf_a if k == 0 else featbuf_b
                for (lo, rows) in ((0, 64), (64, 64), (128, rk - 128)):
                    P = 2 * rows
                    ro = (2 * k + lo) * W          # field row offset
                    slab = slabp.tile([128, 5, 612], BF16, tag="slab")
                    for h in range(2):
                        nc.sync.dma_start(
                            out=slab[h * rows:(h + 1) * rows],
                            in_=_dap(src_d, (lo + 2 * k) * WF + W2 * h,
                                     [[WF, rows], [WF, 5], [1, 612]]))
                    # slab1[c] = slab[c+1]: 4B-aligned mirror for odd column shifts
                    slab1 = slabp.tile([128, 5, 612], BF16, tag="slab1")
                    nc.sync.dma_start(out=slab1[0:P, :, 0:611], in_=slab[0:P, :, 1:612])

                    # field loads
                    def ldfield(dst, base_d, off0, nf=1):
                        for h in range(2):
                            dims = [[W, rows]] + ([[FS, nf]] if nf > 1 else []) + [[1, W2]]
                            nc.sync.dma_start(
                                out=dst[h * rows:(h + 1) * rows],
                                in_=_dap(base_d, off0 + ro + W2 * h, dims))

                    omc_t = fldp.tile([128, W2], BF16, tag="omc", bufs=1)
                    ldfield(omc_t, omc_d, 0)
                    cff_t = fldp.tile([128, W2], BF16, tag="cff", bufs=1)
                    ldfield(cff_t, cff_d, 0)
                    offt = fldp.tile([128, 18, W2], BF16, tag="offt")
                    ldfield(offt, offs_d, k * 18 * FS, nf=18)
                    est = fldp.tile([128, 10, W2], BF16, tag="est", bufs=1)
                    ldfield(est, es_d, k * 10 * FS, nf=10)

                    # x-differences (both column parities)
                    dpa = dpp.tile([128, 5, 612], BF16, tag="dpa")
                    dpb = dpp.tile([128, 5, 612], BF16, tag="dpb")
                    nc.vector.tensor_tensor(out=dpa[0:P, :, 0:611], in0=slab1[0:P, :, 0:611],
                                            in1=slab[0:P, :, 0:611], op=ALU.subtract)
                    nc.vector.tensor_tensor(out=dpb[0:P, :, 0:610], in0=slab[0:P, :, 2:612],
                                            in1=slab1[0:P, :, 0:610], op=ALU.subtract)
                    d2a = dpp.tile([128, 5, 612], BF16, tag="d2a")
                    d2b = dpp.tile([128, 5, 612], BF16, tag="d2b")
                    nc.vector.tensor_tensor(out=d2a[0:P, :, 2:611], in0=dpa[0:P, :, 2:611],
                                            in1=dpb[0:P, :, 0:609], op=ALU.subtract)
                    nc.vector.tensor_tensor(out=d2b[0:P, :, 0:610], in0=dpb[0:P, :, 0:610],
                                            in1=dpa[0:P, :, 0:610], op=ALU.subtract)

                    # relu fields on ACT: rdy9[t]=relu(dy_t), rdx9[t]=relu(dx_t)
                    rdy9 = scrp.tile([128, 9, W2], BF16, tag="rdy9")
                    nc.vector.tensor_scalar(
                        out=rdy9[0:P], in0=_svp(offt, P, 0, [[2 * W2, 9], [1, W2]]),
                        scalar1=0.0, scalar2=None, op0=ALU.max)
                    rdx9 = scrp.tile([128, 9, W2], BF16, tag="rdx9")
                    nc.vector.tensor_scalar(
                        out=rdx9[0:P], in0=_svp(offt, P, W2, [[2 * W2, 9], [1, W2]]),
                        scalar1=0.0, scalar2=None, op0=ALU.max)

                    # ---- G stage: per kx, batched over (ky, r) ----
                    # G[kx, ky, r] = slab(ky+r, b) + dx*Dp(b-1) + rdx*D2(b)
                    Gt = gtp.tile([128, 3, 3, 3, W2], BF16, tag="Gt")  # [kx][ky][r]
                    t2t = None
                    t2ts = [scrp.tile([128, 3, 3, W2], BF16, tag=f"t2t{j}", name=f"t2t{j}")
                            for j in range(2)]
                    for kx in range(3):
                        t2t = t2ts[kx % 2]
                        bb = kx - 1
                        sl_t, sl_o = (slab, 2 + bb) if (2 + bb) % 2 == 0 else (slab1, 1 + bb)
                        dp_t, dp_o = (dpa, 1 + bb) if (1 + bb) % 2 == 0 else (dpb, bb)
                        d2_t, d2_o = (d2a, 2 + bb) if (2 + bb) % 2 == 0 else (d2b, 1 + bb)
                        kyr = [[612, 3], [612, 3], [1, W2]]
                        slv = _svp(sl_t, P, sl_o, kyr)
                        dpv = _svp(dp_t, P, dp_o, kyr)
                        d2v = _svp(d2_t, P, d2_o, kyr)
                        dxv = _svp(offt, P, (2 * kx + 1) * W2,
                                   [[6 * W2, 3], [0, 3], [1, W2]])
                        rxv = _svp(rdx9, P, kx * W2,
                                   [[3 * W2, 3], [0, 3], [1, W2]])
                        gv = _svp(Gt, P, kx * 9 * W2,
                                  [[3 * W2, 3], [W2, 3], [1, W2]])
                        t2v = _svp(t2t, P, 0, [[3 * W2, 3], [W2, 3], [1, W2]])
                        teng = nc.gpsimd if kx < 2 else nc.vector
                        nc.vector.tensor_tensor(out=gv, in0=rxv, in1=d2v, op=ALU.mult)
                        teng.tensor_tensor(out=t2v, in0=dxv, in1=dpv, op=ALU.mult)
                        nc.vector.tensor_tensor(out=gv, in0=gv, in1=t2v, op=ALU.add)
                        nc.vector.tensor_tensor(out=gv, in0=gv, in1=slv, op=ALU.add)

                    # ---- y combine: batched over all 9 taps (kx, ky) ----
                    # val = G1 + dy*(G1-G0) + rdy*((G2-G1)-(G1-G0))
                    dY = t2ts[0]      # t2t0 is dead after the G stage
                    T2 = scrp.tile([128, 9, W2], BF16, tag="T2")
                    val = rdx9        # rdx9 is dead after the G stage
                    kk = [[9 * W2, 3], [3 * W2, 3], [1, W2]]   # (kx, ky) dims on Gt
                    g0 = _svp(Gt, P, 0, kk)
                    g1 = _svp(Gt, P, W2, kk)
                    g2 = _svp(Gt, P, 2 * W2, kk)
                    dyv = _svp(offt, P, 0, [[2 * W2, 3], [6 * W2, 3], [1, W2]])
                    ryv = _svp(rdy9, P, 0, [[W2, 3], [3 * W2, 3], [1, W2]])
                    flat9 = [[W2, 9], [1, W2]]
                    dYv = _svp(dY, P, 0, [[3 * W2, 3], [W2, 3], [1, W2]])
                    T2v = _svp(T2, P, 0, [[3 * W2, 3], [W2, 3], [1, W2]])
                    valv = _svp(val, P, 0, [[3 * W2, 3], [W2, 3], [1, W2]])
                    nc.vector.tensor_tensor(out=dYv, in0=g1, in1=g0, op=ALU.subtract)
                    nc.vector.tensor_tensor(out=T2v, in0=g2, in1=g1, op=ALU.subtract)
                    nc.vector.tensor_tensor(out=T2v, in0=T2v, in1=dYv, op=ALU.subtract)
                    nc.vector.tensor_tensor(out=dYv, in0=dyv, in1=dYv, op=ALU.mult)
                    nc.vector.tensor_tensor(out=T2v, in0=ryv, in1=T2v, op=ALU.mult)
                    nc.vector.tensor_tensor(out=valv, in0=g1, in1=dYv, op=ALU.add)
                    nc.vector.tensor_tensor(out=valv, in0=valv, in1=T2v, op=ALU.add)
                    # ev = e * val  (e reordered to (kx, ky) to match val)
                    ev = _svp(est, P, 0, [[W2, 3], [3 * W2, 3], [1, W2]])
                    nc.gpsimd.tensor_tensor(out=valv, in0=ev, in1=valv, op=ALU.mult)

                    # tap accumulation tree (9 -> 1) + restart term
                    s1 = T2[:, 0:4]   # T2 is dead after the y stage
                    nc.vector.tensor_tensor(out=s1[0:P], in0=val[0:P, 0:4],
                                            in1=val[0:P, 4:8], op=ALU.add)
                    nc.vector.tensor_tensor(out=s1[0:P, 0:2], in0=s1[0:P, 0:2],
                                            in1=s1[0:P, 2:4], op=ALU.add)
                    prop = accp.tile([128, W2], BF16, tag="prop")
                    nc.vector.tensor_tensor(out=prop[0:P], in0=s1[0:P, 0],
                                            in1=s1[0:P, 1], op=ALU.add)
                    nc.vector.tensor_tensor(out=prop[0:P], in0=prop[0:P],
                                            in1=val[0:P, 8], op=ALU.add)
                    # restart: += e9 * feat_center
                    tfe = scrp.tile([128, W2], BF16, tag="tfe")
                    nc.gpsimd.tensor_tensor(out=tfe[0:P], in0=est[0:P, 9],
                                            in1=slab[0:P, 2, 2:2 + W2], op=ALU.mult)
                    nc.vector.tensor_tensor(out=prop[0:P], in0=prop[0:P],
                                            in1=tfe[0:P], op=ALU.add)

                    # softmax denominator S = sum est[0:10]; scale = omc/S
                    s5 = T2[:, 4:9]
                    nc.gpsimd.tensor_tensor(out=s5[0:P], in0=est[0:P, 0:5],
                                            in1=est[0:P, 5:10], op=ALU.add)
                    nc.gpsimd.tensor_tensor(out=s5[0:P, 0:2], in0=s5[0:P, 0:2],
                                            in1=s5[0:P, 2:4], op=ALU.add)
                    nc.gpsimd.tensor_tensor(out=s5[0:P, 0], in0=s5[0:P, 0],
                                            in1=s5[0:P, 1], op=ALU.add)
                    nc.gpsimd.tensor_tensor(out=s5[0:P, 0], in0=s5[0:P, 0],
                                            in1=s5[0:P, 4], op=ALU.add)
                    rs_t = scrp.tile([128, W2], F32, tag="rs")
                    nc.vector.reciprocal(out=rs_t[0:P], in_=s5[0:P, 0])
                    omcrs = scrp.tile([128, W2], BF16, tag="omcrs")
                    nc.gpsimd.tensor_tensor(out=omcrs[0:P], in0=omc_t[0:P],
                                            in1=rs_t[0:P], op=ALU.mult)

                    # blend: fnew = prop*omc/S + conf*feat_fix
                    nc.vector.tensor_tensor(out=prop[0:P], in0=prop[0:P],
                                            in1=omcrs[0:P], op=ALU.mult)
                    fnew = accp.tile([128, W2], F32 if k == 2 else BF16,
                                     tag="fnew32" if k == 2 else "fnew16")
                    nc.vector.tensor_tensor(out=fnew[0:P], in0=prop[0:P],
                                            in1=cff_t[0:P], op=ALU.add)
                    for h in range(2):
                        if k < 2:
                            dst = _dap(dst_fb, (2 + 2 * k + lo) * WF + 2 + W2 * h,
                                       [[WF, rows], [1, W2]])
                        else:
                            dst = _dap(out_d, lo * W + W2 * h, [[W, rows], [1, W2]])
                        nc.sync.dma_start(out=dst, in_=fnew[h * rows:(h + 1) * rows])

    nc.compile()
    return nc


def _prep_inputs(inputs):
    """Full inputs -> list of 8 per-core input dicts (host-side shard+pad)."""
    feat_init = np.asarray(inputs["feat_init"], np.float32)
    guidance = np.asarray(inputs["guidance"], np.float32)
    confidence = np.asarray(inputs["confidence"], np.float32)
    feat_fix = np.asarray(inputs["feat_fix"], np.float32)
    W_conv = np.asarray(inputs["W_conv"], np.float32)
    b_conv = np.asarray(inputs["b_conv"], np.float32)

    # channel reorder: original channel o -> (k = o//28, idx = o%28)
    perm_m = np.zeros(84, np.int64)
    bias94 = np.zeros((MM, 1), np.float32)
    for o in range(84):
        k, idx = o // 28, o % 28
        if idx < 18:
            pos = 18 * k + idx
            m = pos if pos < 32 else pos + 3
        else:
            m = 64 + 10 * k + (idx - 18)
        perm_m[o] = m
        bias94[m, 0] = b_conv[o]
    w3 = np.zeros((90, 3, MM), np.float32)
    for o in range(84):
        for c in range(30):
            for ky in range(3):
                for kx in range(3):
                    w3[kx * 30 + c, ky, perm_m[o]] = W_conv[o, c, ky, kx]
    w3 = w3.astype(ml_dtypes.bfloat16)
    conf = np.sign(feat_fix) * (1.0 / (1.0 + np.exp(-confidence)))
    omc_full = (1.0 - conf)[:, 0].astype(np.float32)     # [B,H,W]
    cff_full = (conf * feat_fix)[:, 0].astype(np.float32)

    def pad_rows(img, lo, hi, fill=0.0):
        """rows [lo, hi) of img [H, ...] with zero padding outside."""
        out = np.full((hi - lo,) + img.shape[1:], fill, img.dtype)
        s0, s1 = max(lo, 0), min(hi, H)
        out[s0 - lo:s1 - lo] = img[s0:s1]
        return out

    in_maps = []
    for core in range(NC):
        b, half = core // 2, core % 2
        r0 = half * HALF
        g_sh = np.zeros((30, GR, WG), np.float32)
        glo, ghi = r0 - 5, r0 + HALF + 5
        s0, s1 = max(glo, 0), min(ghi, H)
        g_sh[:, s0 - glo:s1 - glo, 1:W + 1] = guidance[b, :, s0:s1, :]
        f_sh = np.zeros((FR, WF), np.float32)
        flo, fhi = r0 - 6, r0 + HALF + 6
        s0, s1 = max(flo, 0), min(fhi, H)
        f_sh[s0 - flo:s1 - flo, 2:W + 2] = feat_init[b, 0, s0:s1, :]
        in_maps.append({
            "g": g_sh.astype(ml_dtypes.bfloat16),
            "w3": w3,
            "b94": bias94,
            "blog": np.ascontiguousarray(bias94[64:94]),
            "finit": f_sh.astype(ml_dtypes.bfloat16),
            "omc": np.ascontiguousarray(pad_rows(omc_full[b], r0 - 4, r0 + HALF + 4)).astype(ml_dtypes.bfloat16),
            "cff": np.ascontiguousarray(pad_rows(cff_full[b], r0 - 4, r0 + HALF + 4)).astype(ml_dtypes.bfloat16),
        })
    return in_maps


def kernel(**inputs) -> np.ndarray:
    if "nc" not in _CACHE:
        _CACHE["nc"] = _build_program()
    nc = _CACHE["nc"]
    in_maps = _prep_inputs(inputs)
    trace = os.environ.get("KERNEL_TRACE", "0") == "1"
    res = run_bass_kernel_spmd(nc, in_maps, core_ids=list(range(NC)), trace=trace)
    _CACHE["last_result"] = res
    out = np.zeros((B, 1, H, W), np.float32)
    for core in range(NC):
        b, half = core // 2, core % 2
        out[b, 0, half * HALF:(half + 1) * HALF, :] = res.results[core]["out"]
    return out


# revision 9
# speedup vs baseline: 1.3306x; 1.0307x over previous
"""Trainium2 Bass kernel for nn_Dynamic_deformable_DySample_restart.

Problem: 3x3 conv (30->84ch) over guidance produces per-pixel offsets +
softmax affinities for 3 iterations of a modulated deformable 3x3 conv
(bilinear sampling) with restart/confidence blending.

Strategy (8 NeuronCores, pure data parallel, one NEFF):
  - shard = (batch b, H-half) -> 8 shards of 176 output rows (+ margins).
  - Phase 1 (PE): conv as 3 accumulating matmuls (K=90 = 30ch x 3kx taps,
    kx realized as column-shifted loads of bf16 guidance), back-to-back to
    keep the PE in its high p-state; PSUM evacuation split across ACT
    (exp of softmax logits, bias folded) and DVE/GPSIMD (offset bias-add),
    fields spilled to internal DRAM as fp16.
  - Phase 2 (DVE+GPSIMD+ACT): per iteration, per 64-row x 2-half band:
    feat as a row-duplicated slab [P, 5, 612] fp16 (halo rows in the free
    dim). Bilinear sample via the 3-candidate hat identity per axis
      G(row) = f + dx*Dp(b-1) + relu(dx)*D2(b)        (x interp)
      val    = G1 + dy*(G1-G0) + relu(dy)*(G2-2G1+G0) (y interp)
    batched over all (ky, r) row-candidates per kx column group as single
    wide instructions ([P, 3, 3, 608] APs with broadcast dims), then the
    y-combine batched over all 9 taps at once ([P, 3, 3, 608]).
    Each wide op is column-split DVE | GPSIMD to balance both engines.
    Everything fp16 (2x DVE mode); relu fields produced on ACT.
  - Zero-padding at image borders carried in the data (host-padded inputs;
    off-image rows masked via om_conf/conf_ff).
"""
import os
import numpy as np
import ml_dtypes
from contextlib import ExitStack

import concourse.bacc as bacc
import concourse.bass as bass
import concourse.tile as tile
import concourse.mybir as mybir
from concourse.bass_utils import run_bass_kernel_spmd

F32 = mybir.dt.float32
F16 = mybir.dt.float16
BF16 = mybir.dt.bfloat16
ALU = mybir.AluOpType
AF = mybir.ActivationFunctionType

# ---------------- geometry ----------------
B, H, W = 4, 352, 1216
HALF = 176               # output rows per core
NC = 8
C0 = HALF + 8            # 184: rows where fields/iter-0 feat are computed
GR = C0 + 2              # 186: guidance rows needed (conv halo)
FR = C0 + 4              # 188: feat rows (init + buffer)
WG = W + 2               # 1218: guidance cols incl conv pad
WF = W + 4               # 1220: feat cols incl +-2 pad
CH = 8                   # conv row-chunk
NCHUNK = C0 // CH        # 23
NT = 19                  # 512-px tiles per chunk (8 rows x 64 cols)
W2 = W // 2              # 608 col half
FS = C0 * W              # field plane stride (184*1216)
SPL = 440                # G/y ops: cols [0,SPL) on DVE, [SPL,608) on GPSIMD

# conv output channel order (M = 94):
#  offsets occupy m 0..31 and 35..56 (pos = 18*k + idx; m = pos if pos<32
#  else pos+3); m 32..34 are junk; m 64..93: logits (exp reads at base 64);
#  m 57..63 pad.
MM = 94

_CACHE = {}


def _dap(t, offset, dims):
    return bass.AP(tensor=t, offset=offset, ap=[list(d) for d in dims])


def _sv(t, off, dims):
    """Custom strided view of an SBUF tile AP; partition dim preserved."""
    return bass.AP(tensor=t.tensor, offset=t.offset + off,
                   ap=[list(t.ap[0])] + [list(d) for d in dims])


def _svp(t, P, off, dims):
    """Like _sv but with partition count P."""
    return bass.AP(tensor=t.tensor, offset=t.offset + off,
                   ap=[[t.ap[0][0], P]] + [list(d) for d in dims])


def _build_program(do_p1=True, do_p2=True):
    nc = bacc.Bacc("TRN2", target_bir_lowering=False, debug=False)

    g_d = nc.dram_tensor("g", [30, GR, WG], BF16, kind="ExternalInput")
    w3_d = nc.dram_tensor("w3", [90, 3, MM], BF16, kind="ExternalInput")
    b94_d = nc.dram_tensor("b94", [MM, 1], F32, kind="ExternalInput")
    blog_d = nc.dram_tensor("blog", [30, 1], F32, kind="ExternalInput")
    fin_d = nc.dram_tensor("finit", [FR, WF], BF16, kind="ExternalInput")
    omc_d = nc.dram_tensor("omc", [C0, W], BF16, kind="ExternalInput")
    cff_d = nc.dram_tensor("cff", [C0, W], BF16, kind="ExternalInput")
    out_d = nc.dram_tensor("out", [HALF, W], F32, kind="ExternalOutput")

    featbuf_a = nc.dram_tensor("featbuf_a", [FR, WF], BF16, kind="Internal")
    featbuf_b = nc.dram_tensor("featbuf_b", [FR, WF], BF16, kind="Internal")
    offs_d = nc.dram_tensor("offs", [3, 18, C0, W], BF16, kind="Internal")
    es_d = nc.dram_tensor("es", [3, 10, C0, W], BF16, kind="Internal")

    with tile.TileContext(nc) as tc, ExitStack() as octx:
        # ---- persistent small tiles ----
        singles = octx.enter_context(tc.tile_pool(name="singles", bufs=1))
        w3_sb = singles.tile([90, 3, MM], BF16, tag="w3")
        nc.sync.dma_start(out=w3_sb, in_=w3_d.ap())
        b94_sb = singles.tile([MM, 1], F32, tag="b94")
        nc.sync.dma_start(out=b94_sb, in_=b94_d.ap())
        blog_sb = singles.tile([30, 1], F32, tag="blog")
        nc.sync.dma_start(out=blog_sb, in_=blog_d.ap())
        zt = singles.tile([1, 2 * FR], BF16, tag="zt")
        nc.vector.memset(zt, 0.0)
        # zero the feat-buffer column pads (rows never write cols [0,2)+[1218,1220))
        for fb in (featbuf_a, featbuf_b):
            nc.sync.dma_start(out=_dap(fb, 0, [[WF, FR], [1, 2]]),
                              in_=zt[:, 0:2 * FR])
            nc.sync.dma_start(out=_dap(fb, W + 2, [[WF, FR], [1, 2]]),
                              in_=zt[:, 0:2 * FR])

        # ================= Phase 1: conv + field extraction =================
        with ExitStack() as ctx:
            g3p = ctx.enter_context(tc.tile_pool(name="g3", bufs=2))
            stp = ctx.enter_context(tc.tile_pool(name="stage", bufs=2))
            pp = ctx.enter_context(tc.tile_pool(name="psA", bufs=6, space="PSUM"))

            for ci in range(NCHUNK if do_p1 else 0):
                g3 = g3p.tile([90, CH + 2, W], BF16, tag="g3")
                for kx in range(3):
                    nc.sync.dma_start(
                        out=g3[30 * kx:30 * kx + 30],
                        in_=_dap(g_d, (ci * CH) * WG + kx,
                                 [[GR * WG, 30], [WG, CH + 2], [1, W]]))
                all_st = stp.tile([57, CH, NT * 64], BF16, tag="all_st")
                e_st = stp.tile([30, CH, NT * 64], BF16, tag="e_st")
                for gb in range(0, NT, 6):
                    tis = list(range(gb, min(gb + 6, NT)))
                    pas = {ti: pp.tile([MM, 512], F32, tag="pa", name=f"pa{ti}")
                           for ti in tis}
                    for ky in range(3):
                        for ti in tis:
                            nc.tensor.matmul(
                                pas[ti][0:MM], w3_sb[:, ky],
                                g3[:, ky:ky + CH, ti * 64:(ti + 1) * 64],
                                start=(ky == 0), stop=(ky == 2))
                    for ti in tis:
                        nc.scalar.activation(
                            out=e_st[:, :, ti * 64:(ti + 1) * 64], in_=pas[ti][64:94],
                            func=AF.Exp, bias=blog_sb, scale=1.0)
                        # offsets(+bias) rows 0..31+35..56 (rows 32..34 junk)
                        nc.scalar.activation(
                            out=all_st[:, :, ti * 64:(ti + 1) * 64], in_=pas[ti][0:57],
                            func=AF.Identity, bias=b94_sb[0:57], scale=1.0)
                # spill chunk fields to DRAM
                ro = ci * CH * W
                nc.sync.dma_start(
                    out=_dap(offs_d, ro, [[FS, 32], [W, CH], [1, W]]), in_=all_st[0:32])
                nc.sync.dma_start(
                    out=_dap(offs_d, 32 * FS + ro, [[FS, 22], [W, CH], [1, W]]),
                    in_=all_st[35:57])
                nc.sync.dma_start(
                    out=_dap(es_d, ro, [[FS, 30], [W, CH], [1, W]]), in_=e_st)

        # ================= Phase 2: deformable iterations =================
        with ExitStack() as ctx:
            slabp = ctx.enter_context(tc.tile_pool(name="slab", bufs=2))
            dpp = ctx.enter_context(tc.tile_pool(name="dp", bufs=1))
            gtp = ctx.enter_context(tc.tile_pool(name="gt", bufs=1))
            fldp = ctx.enter_context(tc.tile_pool(name="fld", bufs=2))
            scrp = ctx.enter_context(tc.tile_pool(name="scr", bufs=1))
            accp = ctx.enter_context(tc.tile_pool(name="acc", bufs=2))

            for k in range(3 if do_p2 else 0):
                rk = C0 - 4 * k
                src_d = fin_d if k == 0 else (featbuf_a if k == 1 else featbuf_b)
                dst_fb = featbuf_a if k == 0 else featbuf_b
                for (lo, rows) in ((0, 64), (64, 64), (128, rk - 128)):
                    P = 2 * rows
                    ro = (2 * k + lo) * W          # field row offset
                    slab = slabp.tile([128, 5, 612], BF16, tag="slab")
                    for h in range(2):
                        nc.sync.dma_start(
                            out=slab[h * rows:(h + 1) * rows],
                            in_=_dap(src_d, (lo + 2 * k) * WF + W2 * h,
                                     [[WF, rows], [WF, 5], [1, 612]]))
                    # slab1[c] = slab[c+1]: 4B-aligned mirror for odd column shifts
                    slab1 = slabp.tile([128, 5, 612], BF16, tag="slab1")
                    nc.sync.dma_start(out=slab1[0:P, :, 0:611], in_=slab[0:P, :, 1:612])

                    # field loads
                    def ldfield(dst, base_d, off0, nf=1):
                        for h in range(2):
                            dims = [[W, rows]] + ([[FS, nf]] if nf > 1 else []) + [[1, W2]]
                            nc.sync.dma_start(
                                out=dst[h * rows:(h + 1) * rows],
                                in_=_dap(base_d, off0 + ro + W2 * h, dims))

                    omc_t = fldp.tile([128, W2], BF16, tag="omc", bufs=1)
                    ldfield(omc_t, omc_d, 0)
                    cff_t = fldp.tile([128, W2], BF16, tag="cff", bufs=1)
                    ldfield(cff_t, cff_d, 0)
                    offt = fldp.tile([128, 18, W2], BF16, tag="offt")
                    ldfield(offt, offs_d, k * 18 * FS, nf=18)
                    est = fldp.tile([128, 10, W2], BF16, tag="est", bufs=1)
                    ldfield(est, es_d, k * 10 * FS, nf=10)

                    # x-differences (both column parities)
                    dpa = dpp.tile([128, 5, 612], BF16, tag="dpa")
                    dpb = dpp.tile([128, 5, 612], BF16, tag="dpb")
                    nc.vector.tensor_tensor(out=dpa[0:P, :, 0:611], in0=slab1[0:P, :, 0:611],
                                            in1=slab[0:P, :, 0:611], op=ALU.subtract)
                    nc.vector.tensor_tensor(out=dpb[0:P, :, 0:610], in0=slab[0:P, :, 2:612],
                                            in1=slab1[0:P, :, 0:610], op=ALU.subtract)
                    d2a = dpp.tile([128, 5, 612], BF16, tag="d2a")
                    d2b = dpp.tile([128, 5, 612], BF16, tag="d2b")
                    nc.vector.tensor_tensor(out=d2a[0:P, :, 2:611], in0=dpa[0:P, :, 2:611],
                                            in1=dpb[0:P, :, 0:609], op=ALU.subtract)
                    nc.vector.tensor_tensor(out=d2b[0:P, :, 0:610], in0=dpb[0:P, :, 0:610],
                                            in1=dpa[0:P, :, 0:610], op=ALU.subtract)

                    # relu fields on ACT: rdy9[t]=relu(dy_t), rdx9[t]=relu(dx_t)
                    rdy9 = scrp.tile([128, 9, W2], BF16, tag="rdy9")
                    nc.vector.tensor_scalar(
                        out=rdy9[0:P], in0=_svp(offt, P, 0, [[2 * W2, 9], [1, W2]]),
                        scalar1=0.0, scalar2=None, op0=ALU.max)
                    rdx9 = scrp.tile([128, 9, W2], BF16, tag="rdx9")
                    nc.vector.tensor_scalar(
                        out=rdx9[0:P], in0=_svp(offt, P, W2, [[2 * W2, 9], [1, W2]]),
                        scalar1=0.0, scalar2=None, op0=ALU.max)

                    # ---- G stage: per kx, batched over (ky, r) ----
                    # G[kx, ky, r] = slab(ky+r, b) + dx*Dp(b-1) + rdx*D2(b)
                    Gt = gtp.tile([128, 3, 3, 3, W2], BF16, tag="Gt")  # [kx][ky][r]
                    t2t = None
                    t2ts = [scrp.tile([128, 3, 3, W2], BF16, tag=f"t2t{j}", name=f"t2t{j}")
                            for j in range(2)]
                    for kx in range(3):
                        t2t = t2ts[kx % 2]
                        bb = kx - 1
                        sl_t, sl_o = (slab, 2 + bb) if (2 + bb) % 2 == 0 else (slab1, 1 + bb)
                        dp_t, dp_o = (dpa, 1 + bb) if (1 + bb) % 2 == 0 else (dpb, bb)
                        d2_t, d2_o = (d2a, 2 + bb) if (2 + bb) % 2 == 0 else (d2b, 1 + bb)
                        kyr = [[612, 3], [612, 3], [1, W2]]
                        slv = _svp(sl_t, P, sl_o, kyr)
                        dpv = _svp(dp_t, P, dp_o, kyr)
                        d2v = _svp(d2_t, P, d2_o, kyr)
                        dxv = _svp(offt, P, (2 * kx + 1) * W2,
                                   [[6 * W2, 3], [0, 3], [1, W2]])
                        rxv = _svp(rdx9, P, kx * W2,
                                   [[3 * W2, 3], [0, 3], [1, W2]])
                        gv = _svp(Gt, P, kx * 9 * W2,
                                  [[3 * W2, 3], [W2, 3], [1, W2]])
                        t2v = _svp(t2t, P, 0, [[3 * W2, 3], [W2, 3], [1, W2]])
                        teng = nc.gpsimd if kx < 2 else nc.vector
                        nc.vector.tensor_tensor(out=gv, in0=rxv, in1=d2v, op=ALU.mult)
                        teng.tensor_tensor(out=t2v, in0=dxv, in1=dpv, op=ALU.mult)
                        nc.vector.tensor_tensor(out=gv, in0=gv, in1=t2v, op=ALU.add)
                        nc.vector.tensor_tensor(out=gv, in0=gv, in1=slv, op=ALU.add)

                    # ---- y combine: batched over all 9 taps (kx, ky) ----
                    # val = G1 + dy*(G1-G0) + rdy*((G2-G1)-(G1-G0))
                    dY = t2ts[0]      # t2t0 is dead after the G stage
                    T2 = scrp.tile([128, 9, W2], BF16, tag="T2")
                    val = rdx9        # rdx9 is dead after the G stage
                    kk = [[9 * W2, 3], [3 * W2, 3], [1, W2]]   # (kx, ky) dims on Gt
                    g0 = _svp(Gt, P, 0, kk)
                    g1 = _svp(Gt, P, W2, kk)
                    g2 = _svp(Gt, P, 2 * W2, kk)
                    dyv = _svp(offt, P, 0, [[2 * W2, 3], [6 * W2, 3], [1, W2]])
                    ryv = _svp(rdy9, P, 0, [[W2, 3], [3 * W2, 3], [1, W2]])
                    flat9 = [[W2, 9], [1, W2]]
                    dYv = _svp(dY, P, 0, [[3 * W2, 3], [W2, 3], [1, W2]])
                    T2v = _svp(T2, P, 0, [[3 * W2, 3], [W2, 3], [1, W2]])
                    valv = _svp(val, P, 0, [[3 * W2, 3], [W2, 3], [1, W2]])
                    nc.vector.tensor_tensor(out=dYv, in0=g1, in1=g0, op=ALU.subtract)
                    nc.vector.tensor_tensor(out=T2v, in0=g2, in1=g1, op=ALU.subtract)
                    nc.vector.tensor_tensor(out=T2v, in0=T2v, in1=dYv, op=ALU.subtract)
                    nc.vector.tensor_tensor(out=dYv, in0=dyv, in1=dYv, op=ALU.mult)
                    nc.vector.tensor_tensor(out=T2v, in0=ryv, in1=T2v, op=ALU.mult)
                    nc.vector.tensor_tensor(out=valv, in0=g1, in1=dYv, op=ALU.add)
                    nc.vector.tensor_tensor(out=valv, in0=valv, in1=T2v, op=ALU.add)
                    # ev = e * val  (e reordered to (kx, ky) to match val)
                    ev = _svp(est, P, 0, [[W2, 3], [3 * W2, 3], [1, W2]])
                    nc.gpsimd.tensor_tensor(out=valv, in0=ev, in1=valv, op=ALU.mult)

                    # tap accumulation tree (9 -> 1) + restart term
                    s1 = T2[:, 0:4]   # T2 is dead after the y stage
                    nc.vector.tensor_tensor(out=s1[0:P], in0=val[0:P, 0:4],
                                            in1=val[0:P, 4:8], op=ALU.add)
                    nc.vector.tensor_tensor(out=s1[0:P, 0:2], in0=s1[0:P, 0:2],
                                            in1=s1[0:P, 2:4], op=ALU.add)
                    prop = accp.tile([128, W2], BF16, tag="prop")
                    nc.vector.tensor_tensor(out=prop[0:P], in0=s1[0:P, 0],
                                            in1=s1[0:P, 1], op=ALU.add)
                    nc.vector.tensor_tensor(out=prop[0:P], in0=prop[0:P],
                                            in1=val[0:P, 8], op=ALU.add)
                    # restart: += e9 * feat_center
                    tfe = scrp.tile([128, W2], BF16, tag="tfe")
                    nc.gpsimd.tensor_tensor(out=tfe[0:P], in0=est[0:P, 9],
                                            in1=slab[0:P, 2, 2:2 + W2], op=ALU.mult)
                    nc.vector.tensor_tensor(out=prop[0:P], in0=prop[0:P],
                                            in1=tfe[0:P], op=ALU.add)

                    # softmax denominator S = sum est[0:10]; scale = omc/S
                    s5 = T2[:, 4:9]
                    nc.gpsimd.tensor_tensor(out=s5[0:P], in0=est[0:P, 0:5],
                                            in1=est[0:P, 5:10], op=ALU.add)
                    nc.gpsimd.tensor_tensor(out=s5[0:P, 0:2], in0=s5[0:P, 0:2],
                                            in1=s5[0:P, 2:4], op=ALU.add)
                    nc.gpsimd.tensor_tensor(out=s5[0:P, 0], in0=s5[0:P, 0],
                                            in1=s5[0:P, 1], op=ALU.add)
                    nc.gpsimd.tensor_tensor(out=s5[0:P, 0], in0=s5[0:P, 0],
                                            in1=s5[0:P, 4], op=ALU.add)
                    rs_t = scrp.tile([128, W2], F32, tag="rs")
                    nc.vector.reciprocal(out=rs_t[0:P], in_=s5[0:P, 0])
                    omcrs = scrp.tile([128, W2], BF16, tag="omcrs")
                    nc.vector.tensor_tensor(out=omcrs[0:P], in0=omc_t[0:P],
                                            in1=rs_t[0:P], op=ALU.mult)

                    # blend: fnew = prop*omc/S + conf*feat_fix
                    nc.vector.tensor_tensor(out=prop[0:P], in0=prop[0:P],
                                            in1=omcrs[0:P], op=ALU.mult)
                    fnew = accp.tile([128, W2], F32 if k == 2 else BF16,
                                     tag="fnew32" if k == 2 else "fnew16")
                    nc.vector.tensor_tensor(out=fnew[0:P], in0=prop[0:P],
                                            in1=cff_t[0:P], op=ALU.add)
                    for h in range(2):
                        if k < 2:
                            dst = _dap(dst_fb, (2 + 2 * k + lo) * WF + 2 + W2 * h,
                                       [[WF, rows], [1, W2]])
                        else:
                            dst = _dap(out_d, lo * W + W2 * h, [[W, rows], [1, W2]])
                        nc.sync.dma_start(out=dst, in_=fnew[h * rows:(h + 1) * rows])

    nc.compile()
    return nc


def _prep_inputs(inputs):
    """Full inputs -> list of 8 per-core input dicts (host-side shard+pad)."""
    feat_init = np.asarray(inputs["feat_init"], np.float32)
    guidance = np.asarray(inputs["guidance"], np.float32)
    confidence = np.asarray(inputs["confidence"], np.float32)
    feat_fix = np.asarray(inputs["feat_fix"], np.float32)
    W_conv = np.asarray(inputs["W_conv"], np.float32)
    b_conv = np.asarray(inputs["b_conv"], np.float32)

    # channel reorder: original channel o -> (k = o//28, idx = o%28)
    perm_m = np.zeros(84, np.int64)
    bias94 = np.zeros((MM, 1), np.float32)
    for o in range(84):
        k, idx = o // 28, o % 28
        if idx < 18:
            pos = 18 * k + idx
            m = pos if pos < 32 else pos + 3
        else:
            m = 64 + 10 * k + (idx - 18)
        perm_m[o] = m
        bias94[m, 0] = b_conv[o]
    w3 = np.zeros((90, 3, MM), np.float32)
    for o in range(84):
        for c in range(30):
            for ky in range(3):
                for kx in range(3):
                    w3[kx * 30 + c, ky, perm_m[o]] = W_conv[o, c, ky, kx]
    w3 = w3.astype(ml_dtypes.bfloat16)
    conf = np.sign(feat_fix) * (1.0 / (1.0 + np.exp(-confidence)))
    omc_full = (1.0 - conf)[:, 0].astype(np.float32)     # [B,H,W]
    cff_full = (conf * feat_fix)[:, 0].astype(np.float32)

    def pad_rows(img, lo, hi, fill=0.0):
        """rows [lo, hi) of img [H, ...] with zero padding outside."""
        out = np.full((hi - lo,) + img.shape[1:], fill, img.dtype)
        s0, s1 = max(lo, 0), min(hi, H)
        out[s0 - lo:s1 - lo] = img[s0:s1]
        return out

    in_maps = []
    for core in range(NC):
        b, half = core // 2, core % 2
        r0 = half * HALF
        g_sh = np.zeros((30, GR, WG), np.float32)
        glo, ghi = r0 - 5, r0 + HALF + 5
        s0, s1 = max(glo, 0), min(ghi, H)
        g_sh[:, s0 - glo:s1 - glo, 1:W + 1] = guidance[b, :, s0:s1, :]
        f_sh = np.zeros((FR, WF), np.float32)
        flo, fhi = r0 - 6, r0 + HALF + 6
        s0, s1 = max(flo, 0), min(fhi, H)
        f_sh[s0 - flo:s1 - flo, 2:W + 2] = feat_init[b, 0, s0:s1, :]
        in_maps.append({
            "g": g_sh.astype(ml_dtypes.bfloat16),
            "w3": w3,
            "b94": bias94,
            "blog": np.ascontiguousarray(bias94[64:94]),
            "finit": f_sh.astype(ml_dtypes.bfloat16),
            "omc": np.ascontiguousarray(pad_rows(omc_full[b], r0 - 4, r0 + HALF + 4)).astype(ml_dtypes.bfloat16),
            "cff": np.ascontiguousarray(pad_rows(cff_full[b], r0 - 4, r0 + HALF + 4)).astype(ml_dtypes.bfloat16),
        })
    return in_maps


def kernel(**inputs) -> np.ndarray:
    if "nc" not in _CACHE:
        _CACHE["nc"] = _build_program()
    nc = _CACHE["nc"]
    in_maps = _prep_inputs(inputs)
    trace = os.environ.get("KERNEL_TRACE", "0") == "1"
    res = run_bass_kernel_spmd(nc, in_maps, core_ids=list(range(NC)), trace=trace)
    _CACHE["last_result"] = res
    out = np.zeros((B, 1, H, W), np.float32)
    for core in range(NC):
        b, half = core // 2, core % 2
        out[b, 0, half * HALF:(half + 1) * HALF, :] = res.results[core]["out"]
    return out


# revision 10
# speedup vs baseline: 1.4470x; 1.0875x over previous
"""Trainium2 Bass kernel for nn_Dynamic_deformable_DySample_restart.

Problem: 3x3 conv (30->84ch) over guidance produces per-pixel offsets +
softmax affinities for 3 iterations of a modulated deformable 3x3 conv
(bilinear sampling) with restart/confidence blending.

Strategy (8 NeuronCores, pure data parallel, one NEFF):
  - shard = (batch b, H-half) -> 8 shards of 176 output rows (+ margins).
  - Phase 1 (PE): conv as 3 accumulating matmuls (K=90 = 30ch x 3kx taps,
    kx realized as column-shifted loads of bf16 guidance), back-to-back to
    keep the PE in its high p-state; PSUM evacuation split across ACT
    (exp of softmax logits, bias folded) and DVE/GPSIMD (offset bias-add),
    fields spilled to internal DRAM as fp16.
  - Phase 2 (DVE+GPSIMD+ACT): per iteration, per 64-row x 2-half band:
    feat as a row-duplicated slab [P, 5, 612] fp16 (halo rows in the free
    dim). Bilinear sample via the 3-candidate hat identity per axis
      G(row) = f + dx*Dp(b-1) + relu(dx)*D2(b)        (x interp)
      val    = G1 + dy*(G1-G0) + relu(dy)*(G2-2G1+G0) (y interp)
    batched over all (ky, r) row-candidates per kx column group as single
    wide instructions ([P, 3, 3, 608] APs with broadcast dims), then the
    y-combine batched over all 9 taps at once ([P, 3, 3, 608]).
    Each wide op is column-split DVE | GPSIMD to balance both engines.
    Everything fp16 (2x DVE mode); relu fields produced on ACT.
  - Zero-padding at image borders carried in the data (host-padded inputs;
    off-image rows masked via om_conf/conf_ff).
"""
import os
import numpy as np
import ml_dtypes
from contextlib import ExitStack

import concourse.bacc as bacc
import concourse.bass as bass
import concourse.tile as tile
import concourse.mybir as mybir
from concourse.bass_utils import run_bass_kernel_spmd

F32 = mybir.dt.float32
F16 = mybir.dt.float16
BF16 = mybir.dt.bfloat16
ALU = mybir.AluOpType
AF = mybir.ActivationFunctionType

# ---------------- geometry ----------------
B, H, W = 4, 352, 1216
HALF = 176               # output rows per core
NC = 8
C0 = HALF + 8            # 184: rows where fields/iter-0 feat are computed
GR = C0 + 2              # 186: guidance rows needed (conv halo)
FR = C0 + 4              # 188: feat rows (init + buffer)
WG = W + 2               # 1218: guidance cols incl conv pad
WF = W + 4               # 1220: feat cols incl +-2 pad
CH = 8                   # conv row-chunk
NCHUNK = C0 // CH        # 23
NT = 19                  # 512-px tiles per chunk (8 rows x 64 cols)
W2 = W // 2              # 608 col half
FS = C0 * W              # field plane stride (184*1216)
SPL = 440                # G/y ops: cols [0,SPL) on DVE, [SPL,608) on GPSIMD

# conv output channel order (M = 94):
#  offsets occupy m 0..31 and 35..56 (pos = 18*k + idx; m = pos if pos<32
#  else pos+3); m 32..34 are junk; m 64..93: logits (exp reads at base 64);
#  m 57..63 pad.
MM = 94

_CACHE = {}


def _dap(t, offset, dims):
    return bass.AP(tensor=t, offset=offset, ap=[list(d) for d in dims])


def _sv(t, off, dims):
    """Custom strided view of an SBUF tile AP; partition dim preserved."""
    return bass.AP(tensor=t.tensor, offset=t.offset + off,
                   ap=[list(t.ap[0])] + [list(d) for d in dims])


def _svp(t, P, off, dims):
    """Like _sv but with partition count P."""
    return bass.AP(tensor=t.tensor, offset=t.offset + off,
                   ap=[[t.ap[0][0], P]] + [list(d) for d in dims])


def _build_program(do_p1=True, do_p2=True):
    nc = bacc.Bacc("TRN2", target_bir_lowering=False, debug=False)

    g_d = nc.dram_tensor("g", [30, GR, WG], BF16, kind="ExternalInput")
    w3_d = nc.dram_tensor("w3", [90, 3, MM], BF16, kind="ExternalInput")
    b94_d = nc.dram_tensor("b94", [MM, 1], F32, kind="ExternalInput")
    blog_d = nc.dram_tensor("blog", [30, 1], F32, kind="ExternalInput")
    fin_d = nc.dram_tensor("finit", [FR, WF], BF16, kind="ExternalInput")
    omc_d = nc.dram_tensor("omc", [C0, W], BF16, kind="ExternalInput")
    cff_d = nc.dram_tensor("cff", [C0, W], BF16, kind="ExternalInput")
    out_d = nc.dram_tensor("out", [HALF, W], F32, kind="ExternalOutput")

    featbuf_a = nc.dram_tensor("featbuf_a", [FR, WF], BF16, kind="Internal")
    featbuf_b = nc.dram_tensor("featbuf_b", [FR, WF], BF16, kind="Internal")
    offs_d = nc.dram_tensor("offs", [3, 18, C0, W], BF16, kind="Internal")
    es_d = nc.dram_tensor("es", [3, 10, C0, W], BF16, kind="Internal")

    with tile.TileContext(nc) as tc, ExitStack() as octx:
        # ---- persistent small tiles ----
        singles = octx.enter_context(tc.tile_pool(name="singles", bufs=1))
        w3_sb = singles.tile([90, 3, MM], BF16, tag="w3")
        nc.sync.dma_start(out=w3_sb, in_=w3_d.ap())
        b94_sb = singles.tile([MM, 1], F32, tag="b94")
        nc.sync.dma_start(out=b94_sb, in_=b94_d.ap())
        blog_sb = singles.tile([30, 1], F32, tag="blog")
        nc.sync.dma_start(out=blog_sb, in_=blog_d.ap())
        zt = singles.tile([1, 2 * FR], BF16, tag="zt")
        nc.vector.memset(zt, 0.0)
        # zero the feat-buffer column pads (rows never write cols [0,2)+[1218,1220))
        for fb in (featbuf_a, featbuf_b):
            nc.sync.dma_start(out=_dap(fb, 0, [[WF, FR], [1, 2]]),
                              in_=zt[:, 0:2 * FR])
            nc.sync.dma_start(out=_dap(fb, W + 2, [[WF, FR], [1, 2]]),
                              in_=zt[:, 0:2 * FR])

        # ================= Phase 1: conv + field extraction =================
        with ExitStack() as ctx:
            g3p = ctx.enter_context(tc.tile_pool(name="g3", bufs=2))
            stp = ctx.enter_context(tc.tile_pool(name="stage", bufs=2))
            pp = ctx.enter_context(tc.tile_pool(name="psA", bufs=6, space="PSUM"))

            for ci in range(NCHUNK if do_p1 else 0):
                g3 = g3p.tile([90, CH + 2, W], BF16, tag="g3")
                for kx in range(3):
                    nc.sync.dma_start(
                        out=g3[30 * kx:30 * kx + 30],
                        in_=_dap(g_d, (ci * CH) * WG + kx,
                                 [[GR * WG, 30], [WG, CH + 2], [1, W]]))
                all_st = stp.tile([57, CH, NT * 64], BF16, tag="all_st")
                e_st = stp.tile([30, CH, NT * 64], BF16, tag="e_st")
                for gb in range(0, NT, 6):
                    tis = list(range(gb, min(gb + 6, NT)))
                    pas = {ti: pp.tile([MM, 512], F32, tag="pa", name=f"pa{ti}")
                           for ti in tis}
                    for ky in range(3):
                        for ti in tis:
                            nc.tensor.matmul(
                                pas[ti][0:MM], w3_sb[:, ky],
                                g3[:, ky:ky + CH, ti * 64:(ti + 1) * 64],
                                start=(ky == 0), stop=(ky == 2))
                    for ti in tis:
                        nc.scalar.activation(
                            out=e_st[:, :, ti * 64:(ti + 1) * 64], in_=pas[ti][64:94],
                            func=AF.Exp, bias=blog_sb, scale=1.0)
                        # offsets(+bias) rows 0..31+35..56 (rows 32..34 junk)
                        nc.scalar.activation(
                            out=all_st[:, :, ti * 64:(ti + 1) * 64], in_=pas[ti][0:57],
                            func=AF.Identity, bias=b94_sb[0:57], scale=1.0)
                # spill chunk fields to DRAM
                ro = ci * CH * W
                nc.sync.dma_start(
                    out=_dap(offs_d, ro, [[FS, 32], [W, CH], [1, W]]), in_=all_st[0:32])
                nc.sync.dma_start(
                    out=_dap(offs_d, 32 * FS + ro, [[FS, 22], [W, CH], [1, W]]),
                    in_=all_st[35:57])
                nc.sync.dma_start(
                    out=_dap(es_d, ro, [[FS, 30], [W, CH], [1, W]]), in_=e_st)

        # ================= Phase 2: deformable iterations =================
        with ExitStack() as ctx:
            slabp = ctx.enter_context(tc.tile_pool(name="slab", bufs=2))
            dpp = ctx.enter_context(tc.tile_pool(name="dp", bufs=1))
            gtp = ctx.enter_context(tc.tile_pool(name="gt", bufs=1))
            fldp = ctx.enter_context(tc.tile_pool(name="fld", bufs=2))
            scrp = ctx.enter_context(tc.tile_pool(name="scr", bufs=1))
            accp = ctx.enter_context(tc.tile_pool(name="acc", bufs=2))

            for k in range(3 if do_p2 else 0):
                rk = C0 - 4 * k
                src_d = fin_d if k == 0 else (featbuf_a if k == 1 else featbuf_b)
                dst_fb = featbuf_a if k == 0 else featbuf_b
                for (lo, rows) in ((0, 64), (64, 64), (128, rk - 128)):
                    P = 2 * rows
                    ro = (2 * k + lo) * W          # field row offset
                    slab = slabp.tile([128, 5, 612], BF16, tag="slab")
                    for h in range(2):
                        nc.sync.dma_start(
                            out=slab[h * rows:(h + 1) * rows],
                            in_=_dap(src_d, (lo + 2 * k) * WF + W2 * h,
                                     [[WF, rows], [WF, 5], [1, 612]]))
                    # slab1[c] = slab[c+1]: 4B-aligned mirror for odd column shifts
                    slab1 = slabp.tile([128, 5, 612], BF16, tag="slab1")
                    nc.sync.dma_start(out=slab1[0:P, :, 0:611], in_=slab[0:P, :, 1:612])

                    # field loads
                    def ldfield(dst, base_d, off0, nf=1):
                        for h in range(2):
                            dims = [[W, rows]] + ([[FS, nf]] if nf > 1 else []) + [[1, W2]]
                            nc.sync.dma_start(
                                out=dst[h * rows:(h + 1) * rows],
                                in_=_dap(base_d, off0 + ro + W2 * h, dims))

                    omc_t = fldp.tile([128, W2], BF16, tag="omc", bufs=1)
                    ldfield(omc_t, omc_d, 0)
                    cff_t = fldp.tile([128, W2], BF16, tag="cff", bufs=1)
                    ldfield(cff_t, cff_d, 0)
                    offt = fldp.tile([128, 18, W2], BF16, tag="offt")
                    ldfield(offt, offs_d, k * 18 * FS, nf=18)
                    est = fldp.tile([128, 10, W2], BF16, tag="est", bufs=1)
                    ldfield(est, es_d, k * 10 * FS, nf=10)

                    # x-differences (both column parities)
                    dpa = dpp.tile([128, 5, 612], BF16, tag="dpa")
                    dpb = dpp.tile([128, 5, 612], BF16, tag="dpb")
                    nc.vector.tensor_tensor(out=dpa[0:P, :, 0:611], in0=slab1[0:P, :, 0:611],
                                            in1=slab[0:P, :, 0:611], op=ALU.subtract)
                    nc.vector.tensor_tensor(out=dpb[0:P, :, 0:610], in0=slab[0:P, :, 2:612],
                                            in1=slab1[0:P, :, 0:610], op=ALU.subtract)
                    d2a = dpp.tile([128, 5, 612], BF16, tag="d2a")
                    d2b = dpp.tile([128, 5, 612], BF16, tag="d2b")
                    nc.vector.tensor_tensor(out=d2a[0:P, :, 2:611], in0=dpa[0:P, :, 2:611],
                                            in1=dpb[0:P, :, 0:609], op=ALU.subtract)
                    nc.vector.tensor_tensor(out=d2b[0:P, :, 0:610], in0=dpb[0:P, :, 0:610],
                                            in1=dpa[0:P, :, 0:610], op=ALU.subtract)

                    # relu fields on ACT: rdy9[t]=relu(dy_t), rdx9[t]=relu(dx_t)
                    rdy9 = scrp.tile([128, 9, W2], BF16, tag="rdy9")
                    nc.vector.tensor_scalar(
                        out=rdy9[0:P], in0=_svp(offt, P, 0, [[2 * W2, 9], [1, W2]]),
                        scalar1=0.0, scalar2=None, op0=ALU.max)
                    rdx9 = scrp.tile([128, 9, W2], BF16, tag="rdx9")
                    nc.vector.tensor_scalar(
                        out=rdx9[0:P], in0=_svp(offt, P, W2, [[2 * W2, 9], [1, W2]]),
                        scalar1=0.0, scalar2=None, op0=ALU.max)

                    # ---- G stage: per kx, batched over (ky, r) ----
                    # G[kx, ky, r] = slab(ky+r, b) + dx*Dp(b-1) + rdx*D2(b)
                    Gt = gtp.tile([128, 3, 3, 3, W2], BF16, tag="Gt")  # [kx][ky][r]
                    t2t = None
                    t2ts = [scrp.tile([128, 3, 3, W2], BF16, tag=f"t2t{j}", name=f"t2t{j}")
                            for j in range(2)]
                    for kx in range(3):
                        t2t = t2ts[kx % 2]
                        bb = kx - 1
                        sl_t, sl_o = (slab, 2 + bb) if (2 + bb) % 2 == 0 else (slab1, 1 + bb)
                        dp_t, dp_o = (dpa, 1 + bb) if (1 + bb) % 2 == 0 else (dpb, bb)
                        d2_t, d2_o = (d2a, 2 + bb) if (2 + bb) % 2 == 0 else (d2b, 1 + bb)
                        kyr = [[612, 3], [612, 3], [1, W2]]
                        slv = _svp(sl_t, P, sl_o, kyr)
                        dpv = _svp(dp_t, P, dp_o, kyr)
                        d2v = _svp(d2_t, P, d2_o, kyr)
                        dxv = _svp(offt, P, (2 * kx + 1) * W2,
                                   [[6 * W2, 3], [0, 3], [1, W2]])
                        rxv = _svp(rdx9, P, kx * W2,
                                   [[3 * W2, 3], [0, 3], [1, W2]])
                        gv = _svp(Gt, P, kx * 9 * W2,
                                  [[3 * W2, 3], [W2, 3], [1, W2]])
                        t2v = _svp(t2t, P, 0, [[3 * W2, 3], [W2, 3], [1, W2]])
                        nc.vector.tensor_tensor(out=gv, in0=rxv, in1=d2v, op=ALU.mult)
                        nc.vector.tensor_tensor(out=t2v, in0=dxv, in1=dpv, op=ALU.mult)
                        nc.vector.tensor_tensor(out=gv, in0=gv, in1=t2v, op=ALU.add)
                        nc.vector.tensor_tensor(out=gv, in0=gv, in1=slv, op=ALU.add)

                    # ---- y combine: batched over all 9 taps (kx, ky) ----
                    # val = G1 + dy*(G1-G0) + rdy*((G2-G1)-(G1-G0))
                    dY = t2ts[0]      # t2t0 is dead after the G stage
                    T2 = scrp.tile([128, 9, W2], BF16, tag="T2")
                    val = rdx9        # rdx9 is dead after the G stage
                    kk = [[9 * W2, 3], [3 * W2, 3], [1, W2]]   # (kx, ky) dims on Gt
                    g0 = _svp(Gt, P, 0, kk)
                    g1 = _svp(Gt, P, W2, kk)
                    g2 = _svp(Gt, P, 2 * W2, kk)
                    dyv = _svp(offt, P, 0, [[2 * W2, 3], [6 * W2, 3], [1, W2]])
                    ryv = _svp(rdy9, P, 0, [[W2, 3], [3 * W2, 3], [1, W2]])
                    flat9 = [[W2, 9], [1, W2]]
                    dYv = _svp(dY, P, 0, [[3 * W2, 3], [W2, 3], [1, W2]])
                    T2v = _svp(T2, P, 0, [[3 * W2, 3], [W2, 3], [1, W2]])
                    valv = _svp(val, P, 0, [[3 * W2, 3], [W2, 3], [1, W2]])
                    nc.vector.tensor_tensor(out=dYv, in0=g1, in1=g0, op=ALU.subtract)
                    nc.vector.tensor_tensor(out=T2v, in0=g2, in1=g1, op=ALU.subtract)
                    nc.vector.tensor_tensor(out=T2v, in0=T2v, in1=dYv, op=ALU.subtract)
                    nc.vector.tensor_tensor(out=dYv, in0=dyv, in1=dYv, op=ALU.mult)
                    nc.vector.tensor_tensor(out=T2v, in0=ryv, in1=T2v, op=ALU.mult)
                    nc.vector.tensor_tensor(out=valv, in0=g1, in1=dYv, op=ALU.add)
                    nc.vector.tensor_tensor(out=valv, in0=valv, in1=T2v, op=ALU.add)
                    # ev = e * val  (e reordered to (kx, ky) to match val)
                    ev = _svp(est, P, 0, [[W2, 3], [3 * W2, 3], [1, W2]])
                    nc.vector.tensor_tensor(out=valv, in0=ev, in1=valv, op=ALU.mult)

                    # tap accumulation tree (9 -> 1) + restart term
                    s1 = T2[:, 0:4]   # T2 is dead after the y stage
                    nc.vector.tensor_tensor(out=s1[0:P], in0=val[0:P, 0:4],
                                            in1=val[0:P, 4:8], op=ALU.add)
                    nc.vector.tensor_tensor(out=s1[0:P, 0:2], in0=s1[0:P, 0:2],
                                            in1=s1[0:P, 2:4], op=ALU.add)
                    prop = accp.tile([128, W2], BF16, tag="prop")
                    nc.vector.tensor_tensor(out=prop[0:P], in0=s1[0:P, 0],
                                            in1=s1[0:P, 1], op=ALU.add)
                    nc.vector.tensor_tensor(out=prop[0:P], in0=prop[0:P],
                                            in1=val[0:P, 8], op=ALU.add)
                    # restart: += e9 * feat_center
                    tfe = scrp.tile([128, W2], BF16, tag="tfe")
                    nc.gpsimd.tensor_tensor(out=tfe[0:P], in0=est[0:P, 9],
                                            in1=slab[0:P, 2, 2:2 + W2], op=ALU.mult)
                    nc.vector.tensor_tensor(out=prop[0:P], in0=prop[0:P],
                                            in1=tfe[0:P], op=ALU.add)

                    # softmax denominator S = sum est[0:10]; scale = omc/S
                    s5 = T2[:, 4:9]
                    nc.gpsimd.tensor_tensor(out=s5[0:P], in0=est[0:P, 0:5],
                                            in1=est[0:P, 5:10], op=ALU.add)
                    nc.gpsimd.tensor_tensor(out=s5[0:P, 0:2], in0=s5[0:P, 0:2],
                                            in1=s5[0:P, 2:4], op=ALU.add)
                    nc.gpsimd.tensor_tensor(out=s5[0:P, 0], in0=s5[0:P, 0],
                                            in1=s5[0:P, 1], op=ALU.add)
                    nc.gpsimd.tensor_tensor(out=s5[0:P, 0], in0=s5[0:P, 0],
                                            in1=s5[0:P, 4], op=ALU.add)
                    rs_t = scrp.tile([128, W2], F32, tag="rs")
                    nc.vector.reciprocal(out=rs_t[0:P], in_=s5[0:P, 0])
                    omcrs = scrp.tile([128, W2], BF16, tag="omcrs")
                    nc.vector.tensor_tensor(out=omcrs[0:P], in0=omc_t[0:P],
                                            in1=rs_t[0:P], op=ALU.mult)

                    # blend: fnew = prop*omc/S + conf*feat_fix
                    nc.vector.tensor_tensor(out=prop[0:P], in0=prop[0:P],
                                            in1=omcrs[0:P], op=ALU.mult)
                    fnew = accp.tile([128, W2], F32 if k == 2 else BF16,
                                     tag="fnew32" if k == 2 else "fnew16")
                    nc.vector.tensor_tensor(out=fnew[0:P], in0=prop[0:P],
                                            in1=cff_t[0:P], op=ALU.add)
                    for h in range(2):
                        if k < 2:
                            dst = _dap(dst_fb, (2 + 2 * k + lo) * WF + 2 + W2 * h,
                                       [[WF, rows], [1, W2]])
                        else:
                            dst = _dap(out_d, lo * W + W2 * h, [[W, rows], [1, W2]])
                        nc.sync.dma_start(out=dst, in_=fnew[h * rows:(h + 1) * rows])

    nc.compile()
    return nc


def _prep_inputs(inputs):
    """Full inputs -> list of 8 per-core input dicts (host-side shard+pad)."""
    feat_init = np.asarray(inputs["feat_init"], np.float32)
    guidance = np.asarray(inputs["guidance"], np.float32)
    confidence = np.asarray(inputs["confidence"], np.float32)
    feat_fix = np.asarray(inputs["feat_fix"], np.float32)
    W_conv = np.asarray(inputs["W_conv"], np.float32)
    b_conv = np.asarray(inputs["b_conv"], np.float32)

    # channel reorder: original channel o -> (k = o//28, idx = o%28)
    perm_m = np.zeros(84, np.int64)
    bias94 = np.zeros((MM, 1), np.float32)
    for o in range(84):
        k, idx = o // 28, o % 28
        if idx < 18:
            pos = 18 * k + idx
            m = pos if pos < 32 else pos + 3
        else:
            m = 64 + 10 * k + (idx - 18)
        perm_m[o] = m
        bias94[m, 0] = b_conv[o]
    w3 = np.zeros((90, 3, MM), np.float32)
    for o in range(84):
        for c in range(30):
            for ky in range(3):
                for kx in range(3):
                    w3[kx * 30 + c, ky, perm_m[o]] = W_conv[o, c, ky, kx]
    w3 = w3.astype(ml_dtypes.bfloat16)
    conf = np.sign(feat_fix) * (1.0 / (1.0 + np.exp(-confidence)))
    omc_full = (1.0 - conf)[:, 0].astype(np.float32)     # [B,H,W]
    cff_full = (conf * feat_fix)[:, 0].astype(np.float32)

    def pad_rows(img, lo, hi, fill=0.0):
        """rows [lo, hi) of img [H, ...] with zero padding outside."""
        out = np.full((hi - lo,) + img.shape[1:], fill, img.dtype)
        s0, s1 = max(lo, 0), min(hi, H)
        out[s0 - lo:s1 - lo] = img[s0:s1]
        return out

    in_maps = []
    for core in range(NC):
        b, half = core // 2, core % 2
        r0 = half * HALF
        g_sh = np.zeros((30, GR, WG), np.float32)
        glo, ghi = r0 - 5, r0 + HALF + 5
        s0, s1 = max(glo, 0), min(ghi, H)
        g_sh[:, s0 - glo:s1 - glo, 1:W + 1] = guidance[b, :, s0:s1, :]
        f_sh = np.zeros((FR, WF), np.float32)
        flo, fhi = r0 - 6, r0 + HALF + 6
        s0, s1 = max(flo, 0), min(fhi, H)
        f_sh[s0 - flo:s1 - flo, 2:W + 2] = feat_init[b, 0, s0:s1, :]
        in_maps.append({
            "g": g_sh.astype(ml_dtypes.bfloat16),
            "w3": w3,
            "b94": bias94,
            "blog": np.ascontiguousarray(bias94[64:94]),
            "finit": f_sh.astype(ml_dtypes.bfloat16),
            "omc": np.ascontiguousarray(pad_rows(omc_full[b], r0 - 4, r0 + HALF + 4)).astype(ml_dtypes.bfloat16),
            "cff": np.ascontiguousarray(pad_rows(cff_full[b], r0 - 4, r0 + HALF + 4)).astype(ml_dtypes.bfloat16),
        })
    return in_maps


def kernel(**inputs) -> np.ndarray:
    if "nc" not in _CACHE:
        _CACHE["nc"] = _build_program()
    nc = _CACHE["nc"]
    in_maps = _prep_inputs(inputs)
    trace = os.environ.get("KERNEL_TRACE", "0") == "1"
    res = run_bass_kernel_spmd(nc, in_maps, core_ids=list(range(NC)), trace=trace)
    _CACHE["last_result"] = res
    out = np.zeros((B, 1, H, W), np.float32)
    for core in range(NC):
        b, half = core // 2, core % 2
        out[b, 0, half * HALF:(half + 1) * HALF, :] = res.results[core]["out"]
    return out
